# revision 1
# baseline (speedup 1.0000x reference)
"""Trainium2 Bass kernel for nn_CompetitiveLayer_2 (competitive equilibrium layer).

Reference computation (per batch row b):
    K = sqrt_K ** 2                                  # (64, 64)
    repeat 30x:  AF = AT / (1 + BF @ K.T);  BF = BT / (1 + AF @ K)
    one more:    AF = AT / (1 + BF @ K.T);  BF = BT / (1 + AF @ K)
    C[b, i, j] = AF[b, i] * K[i, j] * BF[b, j]       # (B, 64, 64)

Sharding: pure data parallel over the batch dim, 1024 rows per core on 8 cores.

Per-core design (cost-model makespan ~95 us; HW scale-rel error ~1.8e-5):
  - State kept TRANSPOSED and 2-group packed: X_T[g*64 + j, col] = X[b, j]
    with b = (2*bl + g)*128 + p, col = bl*128 + p.  Both 64-row groups live in
    one 128-partition tile so PE/ACT/DVE run full width; the group-local
    matmul uses a block-diagonal [128, 128] stationary operand.
  - Each update is a serial chain (PE matmul -> ScalarE reciprocal LUT with
    bias=1 -> DVE multiply), so the 512 batch columns split into M_CHAINS
    independent chains that pipeline across engines.  Emission is
    step-interleaved (all chains' A-steps, then all B-steps) because the
    per-engine sequencers execute in order.  Steady state is bound by the
    ScalarE reciprocal throughput (~2.4 us/round).
  - Rounds: A_PRE plain rounds, then a guarded per-chain Aitken delta^2
    extrapolation (error ~0.56^2k: equals ~19-20 plain rounds), then the
    final differentiable iterate.  End-to-end error is dominated by the
    ScalarE reciprocal LUT (~1.2e-5), same as running the reference's 30
    rounds with that LUT.
  - C phase: Q[b, (i,j)] = BF*[b,j]*K[i,j] via matmuls against a
    diagonally-expanded K (rq[j', i*64+j] = K[i,j] if j==j'), computed as a
    3-term fp32r product (operands split into fp32r-rounded + residual
    parts; fp32r streams 4x faster than fp32 and multiplies its rounded
    operands exactly, so the split is accurate to ~1e-7).  Then one DVE
    multiply by AF*[b,i] (free-dim broadcast) and a 512 KB DMA per quarter
    chunk.  The phase runs at the DMA write floor (~360 GB/s per core,
    16 MB of C per core -> ~47 us).
  - AF* in batch layout comes from small per-chunk matmuls (lhsT = BF
    entering the final round) emitted between the final A- and B-steps.
"""

from contextlib import ExitStack

import numpy as np

import concourse.bass as bass
import concourse.tile as tile
from concourse import bacc, mybir
from concourse.bass_utils import run_bass_kernel_spmd
from concourse.masks import make_identity

F32 = mybir.dt.float32
F32R = mybir.dt.float32r
RECIP = mybir.ActivationFunctionType.Reciprocal


def _act_recip(nc, out, in_, bias=1.0):
    """out = 1 / (in_ + bias) on ScalarE.

    Emits InstActivation directly: nc.scalar.activation() refuses Reciprocal
    because of its LUT accuracy (~1.2e-5 rel, HW-measured), which is fine for
    this kernel's domain (inputs in [1, 22]) and tolerance.
    """
    eng = nc.scalar
    ins = [eng.lower_ap(in_)]
    for arg in (bias, 1.0, 0.0):  # bias, scale, alpha
        ins.append(mybir.ImmediateValue(dtype=mybir.dt.float32, value=float(arg)))
    return eng.add_instruction(
        mybir.InstActivation(
            name=nc.get_next_instruction_name(),
            func=RECIP,
            ins=ins,
            outs=[eng.lower_ap(out)],
        )
    )

P = 128          # SBUF partitions
NA = 64          # AF feature dim (i)
NB = 64          # BF feature dim (j)
B_TOTAL = 8192
N_CORES = 8
B_CORE = B_TOTAL // N_CORES          # 1024
N_CHUNK = B_CORE // P                # 8 output chunks of 128 rows
GROUPS = 2                           # partition-packing groups
COLS = B_CORE // GROUPS              # 512 batch columns per group
N_SOLVE = 18                         # plain solver iterations when AITKEN off
AITKEN = True                        # Aitken delta^2: A_PRE rounds + extrapolate + A_POST
A_PRE = 9                            # plain rounds before extrapolation
A_POST = 0                           # plain rounds after extrapolation
M_CHAINS = 4                         # independent pipeline chains
FD = COLS // M_CHAINS                # free dim per chain (128)


def _emit_core(ctx, tc, at, bt, sqk, c_out, n_solve, m_chains, aitken):
    """Emit the per-core kernel body into TileContext tc.

    at, bt: DRAM APs [1024, 64]; sqk: [64, 64]; c_out: [1024, 4096].
    """
    nc = tc.nc
    fd = COLS // m_chains
    if aitken:
        n_pre, n_post = A_PRE, A_POST
        n_rounds = n_pre + n_post + 1  # +1 = the final differentiable iterate
    else:
        n_pre = None
        n_rounds = n_solve + 1
    bpc = fd // P  # 128-col blocks per chain

    def chunk_map(cc):
        # chunk cc of 128 batch rows -> (group half, col block, chain, col off)
        # g = cc %% 2 keeps each chain's two chunks adjacent in the batch, so
        # the first input-DMA half already covers whole chains.
        g, bl = cc % GROUPS, cc // GROUPS
        return g, bl // bpc, (bl % bpc) * P

    singles = ctx.enter_context(tc.tile_pool(name="singles", bufs=1))
    ps_pool = ctx.enter_context(tc.tile_pool(name="ps", bufs=4, space="PSUM"))
    q_pool = ctx.enter_context(tc.tile_pool(name="qps", bufs=2, space="PSUM"))
    r_pool = ctx.enter_context(tc.tile_pool(name="rp", bufs=8))
    c_pool = ctx.enter_context(tc.tile_pool(name="cp", bufs=6))

    # ---- static tiles -------------------------------------------------
    ident = singles.tile([P, P], F32, tag="ident")
    make_identity(nc, ident)

    at_b = singles.tile([P, COLS], F32, tag="at_b")   # batch layout: free=(chunk, i)
    bt_b = singles.tile([P, COLS], F32, tag="bt_b")
    # transposed 2-group packed inputs, one tile per chain so each chain can
    # start iterating as soon as its own chunks are transposed
    at_tc = [
        singles.tile([P, fd], F32, name=f"at_t{t}", tag=f"at_t{t}")
        for t in range(m_chains)
    ]
    bt_tc = [
        singles.tile([P, fd], F32, name=f"bt_t{t}", tag=f"bt_t{t}")
        for t in range(m_chains)
    ]

    sk = singles.tile([NA, NB], F32, tag="sk")
    kk = singles.tile([NA, NB], F32, tag="kk")        # K = sqrt_K^2   [i, j]
    kt = singles.tile([NB, NA], F32, tag="kt")        # K^T            [j, i]
    w_a = singles.tile([P, P], F32, tag="w_a")        # blockdiag(K, K)
    w_b = singles.tile([P, P], F32, tag="w_b")        # blockdiag(K^T, K^T)
    kt2 = singles.tile([P, NA], F32, tag="kt2")       # K^T in both halves
    kt_r = singles.tile([NB, NA], F32R, tag="kt_r")
    kt_res_f = singles.tile([NB, NA], F32, tag="kt_res_f")
    kt_res = singles.tile([NB, NA], F32R, tag="kt_res")
    rqr = singles.tile([P, NA * NB], F32R, tag="rqr")    # diag_j-expand pieces
    rqres = singles.tile([P, NA * NB], F32R, tag="rqres")

    af_c = [singles.tile([P, fd], F32, name=f"af{t}", tag=f"af{t}") for t in range(m_chains)]
    bf_c = [singles.tile([P, fd], F32, name=f"bf{t}", tag=f"bf{t}") for t in range(m_chains)]
    bfr_c = [
        singles.tile([P, fd], F32R, name=f"bfr{t}", tag=f"bfr{t}")
        for t in range(m_chains)
    ]
    bfe_f = [
        singles.tile([P, fd], F32, name=f"bfef{t}", tag=f"bfef{t}")
        for t in range(m_chains)
    ]
    bfe_c = [
        singles.tile([P, fd], F32R, name=f"bfe{t}", tag=f"bfe{t}")
        for t in range(m_chains)
    ]
    afs_c = [singles.tile([P, NA], F32, name=f"afs{cc}", tag=f"afs{cc}") for cc in range(N_CHUNK)]

    if aitken:
        # Per-chain BF history over the last three pre-rounds + extrapolation
        # scratch, so each chain extrapolates and resumes independently.
        def tiles(pfx, n=m_chains):
            return [
                singles.tile([P, fd], F32, name=f"{pfx}{t}", tag=f"{pfx}{t}")
                for t in range(n)
            ]

        h0_c, h1_c, h2_c = tiles("h0"), tiles("h1"), tiles("h2")
        bfx_c = tiles("bfx")
        akd1_c, akd2_c, akdn_c, aks_c = (
            tiles("akd1"), tiles("akd2"), tiles("akdn"), tiles("aks"),
        )
        hist = {n_pre - 3: h0_c, n_pre - 2: h1_c, n_pre - 1: h2_c}
    else:
        hist = {}

    def bf_read(s, t):
        # BF state entering round s's A-step for chain t
        if s == 0:
            return bt_tc[t]
        if aitken and s == n_pre:
            return bfx_c[t]
        if (s - 1) in hist:
            return hist[s - 1][t]
        return bf_c[t]

    def bf_write(s, t):
        # tile the B-step of round s writes for chain t
        if s in hist:
            return hist[s][t]
        return bf_c[t]

    # ---- load inputs --------------------------------------------------
    # sqrt_K first: the iteration weights are on the critical path.
    # at_b[p, c*64 + i] = AT[c*128 + p, i]; two halves so early chunks land
    # (and their chains start) before the full input is in.
    nc.sync.dma_start(out=sk, in_=sqk)
    at3 = at.rearrange("(c p) i -> p c i", p=P)
    bt3 = bt.rearrange("(c p) i -> p c i", p=P)
    hc = N_CHUNK // 2
    for hh in range(2):
        csl = slice(hh * hc, (hh + 1) * hc)
        nc.sync.dma_start(
            out=at_b.rearrange("p (c i) -> p c i", i=NA)[:, csl, :],
            in_=at3[:, csl, :],
        )
        nc.sync.dma_start(
            out=bt_b.rearrange("p (c i) -> p c i", i=NB)[:, csl, :],
            in_=bt3[:, csl, :],
        )

    # ---- build K, K^T, weights ---------------------------------------
    nc.vector.tensor_mul(kk, sk, sk)
    tp_kt = ps_pool.tile([NB, NA], F32, tag="ps")
    nc.tensor.transpose(tp_kt, kk, ident[0:NA, 0:NA])
    nc.scalar.copy(out=kt, in_=tp_kt)

    nc.vector.memset(w_a, 0.0)
    nc.vector.memset(w_b, 0.0)
    nc.vector.tensor_copy(out=w_a[0:NA, 0:NB], in_=kk)
    nc.vector.tensor_copy(out=w_b[0:NB, 0:NA], in_=kt)
    # second diagonal block: SBUF->SBUF DMA handles the partition shift
    nc.sync.dma_start(out=w_a[NA:P, NB : 2 * NB], in_=kk)
    nc.sync.dma_start(out=w_b[NB:P, NA : 2 * NA], in_=kt)
    nc.vector.tensor_copy(out=kt2[0:NB, :], in_=kt)
    nc.sync.dma_start(out=kt2[NB:P, :], in_=kt)

    # The C-phase expand runs as a 3-term fp32r matmul (1 cyc/row vs 4 for
    # fp32): Q = bf_r*rq_r + bf_r*rq_res + bf_res*rq_r with _r = value
    # rounded to fp32r's mantissa and _res the remainder, exact to ~1e-7
    # (HW-validated).  Round K^T once, then diag-expand both pieces:
    # rq*[j', i*64 + j] = piece[i, j] if j == j' else 0.
    nc.vector.tensor_copy(out=kt_r, in_=kt)
    nc.vector.tensor_sub(out=kt_res_f, in0=kt, in1=kt_r.bitcast(F32))
    nc.vector.tensor_copy(out=kt_res, in_=kt_res_f)
    for src, dst in ((kt_r, rqr), (kt_res, rqres)):
        nc.gpsimd.affine_select(
            out=dst[0:NB, :].rearrange("p (i j) -> p i j", i=NA),
            in_=src[:, :, None].broadcast_to([NB, NA, NB]),
            compare_op=mybir.AluOpType.is_equal,
            fill=0.0,
            base=0,
            pattern=[[0, NA], [1, NB]],
            channel_multiplier=-1,
        )
        nc.sync.dma_start(out=dst[NB:P, :], in_=dst[0:NB, :])

    # ---- transpose AT, BT into 2-group packed layout ------------------
    for cc in range(N_CHUNK):
        g, t, col = chunk_map(cc)
        tp1 = ps_pool.tile([NA, P], F32, tag="ps")
        nc.tensor.transpose(tp1, at_b[:, cc * NA : (cc + 1) * NA], ident)
        nc.scalar.copy(out=at_tc[t][g * NA : (g + 1) * NA, col : col + P], in_=tp1)
        tp2 = ps_pool.tile([NB, P], F32, tag="ps")
        nc.tensor.transpose(tp2, bt_b[:, cc * NB : (cc + 1) * NB], ident)
        nc.vector.tensor_copy(
            out=bt_tc[t][g * NB : (g + 1) * NB, col : col + P], in_=tp2
        )

    # ---- fixed-point iterations --------------------------------------
    # Step-interleaved emission: all chains' A-steps, then all B-steps.
    # Per-engine sequencers execute in order, so chain t's B-matmul must not
    # sit ahead of chain t+1's A-matmul in PE program order.
    for s in range(n_rounds):
        if aitken and s == n_pre:
            # BF* ~= b2 - d2^2 * den / (den^2 + eps), den = d2 - d1.  The eps
            # form is smooth at den -> 0 and needs no predication.  den is
            # pre-scaled by kappa so the ScalarE reciprocal input
            # (kappa^2 den^2 + 1e-12) stays inside its +-[2^-42, 2^42] domain;
            # effective eps = 1e-12/kappa^2 ~ 9e-25, suppressing corrections
            # only where |den| < 1e-12 (already converged).
            kap = float(2 ** 20)
            for t in range(m_chains):
                d1, d2 = akd1_c[t], akd2_c[t]
                dn, sA = akdn_c[t], aks_c[t]
                nc.vector.tensor_sub(out=d1, in0=h1_c[t], in1=h0_c[t])
                nc.vector.tensor_sub(out=d2, in0=h2_c[t], in1=h1_c[t])
                nc.vector.tensor_sub(out=dn, in0=d2, in1=d1)
                nc.vector.tensor_scalar_mul(out=dn, in0=dn, scalar1=kap)
                nc.vector.tensor_mul(sA, dn, dn)
                _act_recip(nc, sA, sA, bias=1e-12)
                nc.vector.tensor_mul(d1, d2, d2)
                nc.vector.tensor_mul(d1, d1, dn)
                nc.vector.tensor_mul(d1, d1, sA)
                nc.vector.tensor_scalar_mul(out=d1, in0=d1, scalar1=kap)
                nc.vector.tensor_sub(out=bfx_c[t], in0=h2_c[t], in1=d1)

        for t in range(m_chains):
            ps1 = ps_pool.tile([P, fd], F32, tag="ps")
            nc.tensor.matmul(ps1, w_b, bf_read(s, t), start=True, stop=True)
            r1 = r_pool.tile([P, fd], F32, tag="r")
            _act_recip(nc, r1, ps1, bias=1.0)
            nc.vector.tensor_mul(af_c[t], at_tc[t], r1)

        if s == n_rounds - 1:
            # AF* in batch layout for the C phase, from BF_{n-1} (the value
            # bf_c[t] still holds -- emitted before the B-step overwrite).
            for cc in range(N_CHUNK):
                g, t, col = chunk_map(cc)
                half = slice(g * NB, (g + 1) * NB)
                coff = slice(col, col + P)
                psb = ps_pool.tile([P, NA], F32, tag="ps")
                nc.tensor.matmul(
                    psb, bf_read(s, t)[half, coff], kt2[half, :],
                    start=True, stop=True,
                )
                rb = r_pool.tile([P, NA], F32, tag="r")
                _act_recip(nc, rb, psb, bias=1.0)
                nc.vector.tensor_mul(
                    afs_c[cc], at_b[:, cc * NA : (cc + 1) * NA], rb
                )

        for t in range(m_chains):
            ps2 = ps_pool.tile([P, fd], F32, tag="ps")
            nc.tensor.matmul(ps2, w_a, af_c[t], start=True, stop=True)
            r2 = r_pool.tile([P, fd], F32, tag="r")
            _act_recip(nc, r2, ps2, bias=1.0)
            nc.vector.tensor_mul(bf_write(s, t), bt_tc[t], r2)
            if s == n_rounds - 1:
                # fp32r split of BF* for the 3-term expand, emitted right
                # after this chain's final B-step so its C chunks start while
                # other chains finish.
                nc.vector.tensor_copy(out=bfr_c[t], in_=bf_c[t])
                nc.vector.tensor_sub(
                    out=bfe_f[t], in0=bf_c[t], in1=bfr_c[t].bitcast(F32)
                )
                nc.vector.tensor_copy(out=bfe_c[t], in_=bfe_f[t])

    # ---- C phase ------------------------------------------------------
    # Q[p, (i,j)] = BF*[b, j] * K[i, j] via 3-term fp32r matmul against the
    # diag-expanded K; C = Q * AF*[b, i] broadcast along j; DMA per quarter.
    NQ = 4          # quarters per chunk
    QW = NA * NB // NQ                   # 1024 elements per quarter
    for cc in range(N_CHUNK):
        g, t, col = chunk_map(cc)
        half = slice(g * NB, (g + 1) * NB)
        coff = slice(col, col + P)
        for q in range(NQ):
            qp = q_pool.tile([P, QW], F32, tag="q")
            for h in range(2):
                nsl = slice(q * QW + h * 512, q * QW + (h + 1) * 512)
                out_sl = qp[:, h * 512 : (h + 1) * 512]
                nc.tensor.matmul(
                    out_sl, bfr_c[t][half, coff], rqr[half, nsl],
                    start=True, stop=False,
                )
                nc.tensor.matmul(
                    out_sl, bfr_c[t][half, coff], rqres[half, nsl],
                    start=False, stop=False,
                )
                nc.tensor.matmul(
                    out_sl, bfe_c[t][half, coff], rqr[half, nsl],
                    start=False, stop=True,
                )
            cs = c_pool.tile([P, QW], F32, tag="c")
            ni = QW // NB                # i-values per quarter (16)
            nc.vector.tensor_mul(
                cs.rearrange("p (i j) -> p i j", i=ni),
                qp.rearrange("p (i j) -> p i j", i=ni),
                afs_c[cc][:, q * ni : (q + 1) * ni][:, :, None].broadcast_to(
                    [P, ni, NB]
                ),
            )
            nc.sync.dma_start(
                out=c_out[cc * P : (cc + 1) * P, q * QW : (q + 1) * QW], in_=cs
            )


def build_nc(n_solve=N_SOLVE, m_chains=M_CHAINS, t_repeat=1, timing_mode=False,
             aitken=None):
    if aitken is None:
        aitken = AITKEN
    nc = bacc.Bacc("TRN2", target_bir_lowering=False, debug=False, num_devices=N_CORES)
    at = nc.dram_tensor("at", (B_CORE, NA), F32, kind="ExternalInput").ap()
    bt = nc.dram_tensor("bt", (B_CORE, NB), F32, kind="ExternalInput").ap()
    sqk = nc.dram_tensor("sqk", (NA, NB), F32, kind="ExternalInput").ap()
    with tile.TileContext(nc) as tc:
        if timing_mode:
            # Write C to internal DRAM scratch; ship back only a tiny token,
            # so wall-clock measurement isn't drowned by the 16 MB/core
            # output transfer through the PJRT tunnel.
            tok = nc.dram_tensor("tok", (1, NA), F32, kind="ExternalOutput").ap()
            with ExitStack() as octx:
                dram = octx.enter_context(
                    tc.tile_pool(name="cdram", bufs=1, space="DRAM")
                )
                c = dram.tile([B_CORE, NA * NB], F32, tag="cscratch")
                for _ in range(t_repeat):
                    with ExitStack() as ctx:
                        _emit_core(ctx, tc, at, bt, sqk, c, n_solve, m_chains, aitken)
                nc.sync.dma_start(out=tok, in_=c[0:1, 0:NA])
        else:
            c = nc.dram_tensor(
                "c", (B_CORE, NA * NB), F32, kind="ExternalOutput"
            ).ap()
            for _ in range(t_repeat):
                with ExitStack() as ctx:
                    _emit_core(ctx, tc, at, bt, sqk, c, n_solve, m_chains, aitken)
    nc.compile()
    return nc


_NC_CACHE = {}


def _get_nc(**kw):
    key = tuple(sorted(kw.items()))
    if key not in _NC_CACHE:
        _NC_CACHE[key] = build_nc(**kw)
    return _NC_CACHE[key]


def kernel(AT, BT, sqrt_K):
    AT = np.ascontiguousarray(AT, dtype=np.float32)
    BT = np.ascontiguousarray(BT, dtype=np.float32)
    sqrt_K = np.ascontiguousarray(sqrt_K, dtype=np.float32)
    nc = _get_nc(n_solve=N_SOLVE, m_chains=M_CHAINS)
    in_maps = [
        {
            "at": AT[c * B_CORE : (c + 1) * B_CORE],
            "bt": BT[c * B_CORE : (c + 1) * B_CORE],
            "sqk": sqrt_K,
        }
        for c in range(N_CORES)
    ]
    res = run_bass_kernel_spmd(nc, in_maps, core_ids=list(range(N_CORES)))
    return np.concatenate(
        [r["c"].reshape(B_CORE, NA, NB) for r in res.results], axis=0
    )



# revision 48
# speedup vs baseline: 1.7296x; 1.7296x over previous
"""Trainium2 Bass kernel for nn_CompetitiveLayer_2 (competitive equilibrium layer).

Reference computation (per batch row b):
    K = sqrt_K ** 2                                  # (64, 64)
    repeat 30x:  AF = AT / (1 + BF @ K.T);  BF = BT / (1 + AF @ K)
    one more:    AF = AT / (1 + BF @ K.T);  BF = BT / (1 + AF @ K)
    C[b, i, j] = AF[b, i] * K[i, j] * BF[b, j]       # (B, 64, 64)

Sharding: pure data parallel over the batch dim, 1024 rows per core on 8 cores.

Per-core design:
  - State kept TRANSPOSED and 2-group packed: X_T[g*64 + j, col] = X[b, j]
    with b = (2*bl + g)*128 + p, col = bl*128 + p.  Both 64-row groups live in
    one 128-partition tile; the group-local matmul uses a block-diagonal
    [128, 128] stationary operand.
  - Each update is a serial chain (PE matmul -> ScalarE reciprocal LUT with
    bias=1 -> DVE multiply); the 512 batch columns split into M_CHAINS
    independent chains that pipeline across engines, step-interleaved.
    A dummy reciprocal at kernel start pulls the ~1.3us ACT table load
    under the input DMA.
  - Rounds: A_PRE plain rounds, then a scalar Richardson extrapolation
    BF* ~= BF_k + GAMMA*(BF_k - BF_{k-1}) (GAMMA = lam/(1-lam) for the
    fixed-point contraction lam~0.52; 2 DVE ops per chain vs 11+recip for
    full Aitken), then the final differentiable A-step.  The final BF* is
    produced per 128-row chunk directly in BATCH layout (psb = AF*@K via
    lhsT = transposed AF* chunk), so the transposed final B-step is
    dropped.  A_PRE=5 + Richardson ~ err 1.4e-3 vs the 2e-2 tolerance;
    the fp32r/fp16 C path adds ~1.5e-3 worst case.
  - C phase (per chunk, per 1024-element quarter): PE computes
    E[b, (i,j)] = AF*[b,i]*K[i,j] as a single fp32r matmul against the
    diag_i-expanded K (ra[i', i*64+j] = K[i,j] if i==i'), then the BF*
    broadcast multiply is spread across three engines to sit at the DMA
    write floor (~23us for 8 MB of fp16 C per core):
      direct quarters:  DVE  cs_fp16 = qp(PSUM f32) * bfs16-broadcast (1x)
      assist quarters:  ACT casts qp -> fp16 SBUF; DVE multiplies at 2x
      pool quarters:    ACT casts qp -> fp16 SBUF; GpSimd multiplies
    (broadcasting BF*[b,j] along i keeps the packed j dim innermost, which
    is what enables the 2x DVE mode).  C is written to DRAM as fp16; the
    host casts back to fp32 on gather.
"""

from contextlib import ExitStack

import numpy as np

import concourse.bass as bass
import concourse.tile as tile
from concourse import bacc, mybir
from concourse.bass_utils import run_bass_kernel_spmd
from concourse.masks import make_identity

F32 = mybir.dt.float32
F32R = mybir.dt.float32r
F16 = mybir.dt.float16
RECIP = mybir.ActivationFunctionType.Reciprocal


def _act_recip(nc, out, in_, bias=1.0):
    """out = 1 / (in_ + bias) on ScalarE.

    Emits InstActivation directly: nc.scalar.activation() refuses Reciprocal
    because of its LUT accuracy (~1.2e-5 rel, HW-measured), which is fine for
    this kernel's domain (inputs in [1, 22]) and tolerance.
    """
    eng = nc.scalar
    ins = [eng.lower_ap(in_)]
    for arg in (bias, 1.0, 0.0):  # bias, scale, alpha
        ins.append(mybir.ImmediateValue(dtype=mybir.dt.float32, value=float(arg)))
    return eng.add_instruction(
        mybir.InstActivation(
            name=nc.get_next_instruction_name(),
            func=RECIP,
            ins=ins,
            outs=[eng.lower_ap(out)],
        )
    )


def _act_copy(nc, out, in_):
    """out = in_ (dtype cast at write) on ScalarE via the Copy LUT."""
    eng = nc.scalar
    ins = [eng.lower_ap(in_)]
    for arg in (0.0, 1.0, 0.0):  # bias, scale, alpha
        ins.append(mybir.ImmediateValue(dtype=mybir.dt.float32, value=float(arg)))
    return eng.add_instruction(
        mybir.InstActivation(
            name=nc.get_next_instruction_name(),
            func=mybir.ActivationFunctionType.Copy,
            ins=ins,
            outs=[eng.lower_ap(out)],
        )
    )


P = 128          # SBUF partitions
NA = 64          # AF feature dim (i)
NB = 64          # BF feature dim (j)
B_TOTAL = 8192
N_CORES = 8
B_CORE = B_TOTAL // N_CORES          # 1024
N_CHUNK = B_CORE // P                # 8 output chunks of 128 rows
GROUPS = 2                           # partition-packing groups
COLS = B_CORE // GROUPS              # 512 batch columns per group
N_SOLVE = 10                         # plain solver iterations when RICH off
RICH = True                          # Richardson extrapolation after A_PRE rounds
A_PRE = 5                            # plain rounds before extrapolation
GAMMA = 1.10                         # Richardson coefficient lam/(1-lam)
M_CHAINS = 4                         # independent pipeline chains
FD = COLS // M_CHAINS                # free dim per chain (128)
# C-phase quarter engine assignment, cycled per (chunk, quarter):
# D = direct DVE (PSUM f32, 1x), A = ACT-cast + 2x DVE, G = ACT-cast + GpSimd
QPAT = ["D", "A", "G", "D"]


def _emit_core(ctx, tc, at, bt, sqk, c_out, n_solve, m_chains, rich,
               qpat=QPAT):
    """Emit the per-core kernel body into TileContext tc.

    at, bt: DRAM APs [1024, 64]; sqk: [64, 64]; c_out: [1024, 4096] fp16.
    """
    nc = tc.nc
    fd = COLS // m_chains
    if rich:
        n_pre = A_PRE
        n_rounds = n_pre + 1  # +1 = the final differentiable A-step
    else:
        n_pre = None
        n_rounds = n_solve + 1
    bpc = fd // P  # 128-col blocks per chain

    def chunk_map(cc):
        # chunk cc of 128 batch rows -> (group half, chain, col off)
        g, bl = cc % GROUPS, cc // GROUPS
        return g, bl // bpc, (bl % bpc) * P

    singles = ctx.enter_context(tc.tile_pool(name="singles", bufs=1))
    # PSUM budget is 8 banks.  One 3-buf pool of 2-bank tiles serves both the
    # iteration matmul outputs and the C-phase qp quarters: 3 bufs is enough
    # for the ACT-saturated iteration (buffer-reuse latency ~830ns < 3 recips
    # = 876ns) and puts the C-phase cadence (~(mm + cast + 2 sems)/3 = 620ns)
    # under the 728ns/quarter DMA floor.  A 2x1-bank aux pool holds the
    # setup transposes and the batch-B psb outputs.
    q_pool = ctx.enter_context(tc.tile_pool(name="qps", bufs=3, space="PSUM"))
    aux_pool = ctx.enter_context(tc.tile_pool(name="aux", bufs=2, space="PSUM"))
    r_pool = ctx.enter_context(tc.tile_pool(name="rp", bufs=8))
    e_pool = ctx.enter_context(tc.tile_pool(name="ep", bufs=6))
    c_pool = ctx.enter_context(tc.tile_pool(name="cp", bufs=10))

    # ---- static tiles -------------------------------------------------
    warm = singles.tile([1, 8], F32, tag="warm")
    # dummy reciprocal: forces the ACT Reciprocal table load at t=0 so the
    # ~1.3us LoadActFuncSet overlaps the input DMA instead of the first round
    nc.vector.memset(warm, 1.0)
    _act_recip(nc, warm, warm, bias=1.0)

    ident = singles.tile([P, P], F32, tag="ident")
    make_identity(nc, ident)

    at_b = singles.tile([P, COLS], F32, tag="at_b")   # batch layout: free=(chunk, i)
    bt_b = singles.tile([P, COLS], F32, tag="bt_b")
    at_tc = [
        singles.tile([P, fd], F32, name=f"at_t{t}", tag=f"at_t{t}")
        for t in range(m_chains)
    ]
    bt_tc = [
        singles.tile([P, fd], F32, name=f"bt_t{t}", tag=f"bt_t{t}")
        for t in range(m_chains)
    ]

    sk2 = singles.tile([P, 2 * NB], F32, tag="sk2")   # sqrt_K in both diag blocks
    kk = singles.tile([NA, NB], F32, tag="kk")        # K = sqrt_K^2   [i, j]
    w_a = singles.tile([P, P], F32, tag="w_a")        # blockdiag(K, K)
    w_b = singles.tile([P, P], F32, tag="w_b")        # blockdiag(K^T, K^T)
    # Richardson folded into the final A-step: w_b @ ((1+g)*BF_k - g*BF_{k-1})
    # as two PSUM-accumulating matmuls against pre-scaled stationaries.
    w_bp = singles.tile([P, P], F32, tag="w_bp")      # (1+GAMMA) * w_b
    w_bm = singles.tile([P, P], F32, tag="w_bm")      # -GAMMA * w_b
    kk2 = singles.tile([P, NB], F32, tag="kk2")       # K in both halves [i, j]
    kk_r = singles.tile([NA, NB], F32R, tag="kk_r")
    ra = singles.tile([P, NA * NB], F32R, tag="ra")   # diag_i-expanded K

    af_c = [singles.tile([P, fd], F32, name=f"af{t}", tag=f"af{t}") for t in range(m_chains)]
    bf_c = [singles.tile([P, fd], F32, name=f"bf{t}", tag=f"bf{t}") for t in range(m_chains)]
    afr_c = [
        singles.tile([P, fd], F32R, name=f"afr{t}", tag=f"afr{t}")
        for t in range(m_chains)
    ]
    bfs16_c = [
        singles.tile([P, NB], F16, name=f"bfs16_{cc}", tag=f"bfs16_{cc}")
        for cc in range(N_CHUNK)
    ]

    if rich:
        h1_c = [
            singles.tile([P, fd], F32, name=f"h1{t}", tag=f"h1{t}")
            for t in range(m_chains)
        ]
        hist = {n_pre - 2: h1_c}
    else:
        hist = {}

    def bf_read(s, t):
        # BF state entering round s's A-step for chain t
        if s == 0:
            return bt_tc[t]
        if (s - 1) in hist:
            return hist[s - 1][t]
        return bf_c[t]

    def bf_write(s, t):
        if s in hist:
            return hist[s][t]
        return bf_c[t]

    # ---- load inputs --------------------------------------------------
    # sqrt_K twice, once per diagonal block, so the block-diagonal weights
    # build with elementwise ops only (no serial SBUF->SBUF partition-shift
    # DMAs on the critical path to round 0).
    at3 = at.rearrange("(c p) i -> p c i", p=P)
    bt3 = bt.rearrange("(c p) i -> p c i", p=P)
    at_bv = at_b.rearrange("p (c i) -> p c i", i=NA)
    bt_bv = bt_b.rearrange("p (c i) -> p c i", i=NB)
    # sqrt_K first (tiny, gates the w_b build), then per-chain-pair input
    # slices with bt of chain t just ahead of at of chain t, so round 0's
    # chains come up at the steady recip cadence instead of waiting for
    # half-batch transfers.
    nc.sync.dma_start(out=sk2[0:NA, 0:NB], in_=sqk)
    nc.sync.dma_start(out=sk2[NA:P, NB : 2 * NB], in_=sqk)
    nbc = N_CHUNK // m_chains
    for t in range(m_chains):
        csl = slice(t * nbc, (t + 1) * nbc)
        nc.sync.dma_start(out=bt_bv[:, csl, :], in_=bt3[:, csl, :])
        nc.sync.dma_start(out=at_bv[:, csl, :], in_=at3[:, csl, :])

    # ---- chain 0's bt transposes + K build, critical-path ordered ------
    # PE FIFO: chain 0's bt transposes first (gated on the bt half-1 DMA),
    # then the w_b transposes (gated on kk <- sk2); everything else defers.
    def tp_chunk(cc, which):
        g, t, col = chunk_map(cc)
        tpi = aux_pool.tile([P, 2 * P], F32, name=f"tp{cc}{which}", tag="aux")
        if which == "b":
            tp2 = tpi[0:NB, 0:P]
            nc.tensor.transpose(tp2, bt_b[:, cc * NB : (cc + 1) * NB], ident)
            nc.vector.tensor_copy(
                out=bt_tc[t][g * NB : (g + 1) * NB, col : col + P], in_=tp2
            )
        else:
            tp1 = tpi[0:NA, P : 2 * P]
            nc.tensor.transpose(tp1, at_b[:, cc * NA : (cc + 1) * NA], ident)
            nc.vector.tensor_copy(
                out=at_tc[t][g * NA : (g + 1) * NA, col : col + P], in_=tp1
            )

    nc.vector.tensor_mul(kk, sk2[0:NA, 0:NB], sk2[0:NA, 0:NB])
    # K^T into both PSUM partition halves via two PE transposes, then copy
    # the diagonal blocks out
    wps = aux_pool.tile([P, 2 * P], F32, tag="aux")
    nc.tensor.transpose(wps[0:NB, 0:NA], kk, ident[0:NA, 0:NA])
    nc.tensor.transpose(wps[NB:P, NA : 2 * NA], kk, ident[0:NA, 0:NA])
    nc.vector.memset(w_b, 0.0)
    nc.vector.tensor_copy(out=w_b[0:NB, 0:NA], in_=wps[0:NB, 0:NA])
    nc.vector.tensor_copy(out=w_b[NB:P, NA : 2 * NA], in_=wps[NB:P, NA : 2 * NA])

    # off the round-0 critical path: B-step / extrapolation / batch-B /
    # C-phase constants
    nc.vector.memset(w_a, 0.0)
    nc.vector.tensor_mul(
        w_a[0:NA, 0:NB], sk2[0:NA, 0:NB], sk2[0:NA, 0:NB]
    )
    nc.vector.tensor_mul(
        w_a[NA:P, NB : 2 * NB], sk2[NA:P, NB : 2 * NB], sk2[NA:P, NB : 2 * NB]
    )
    if rich:
        nc.vector.tensor_scalar_mul(out=w_bp, in0=w_b, scalar1=1.0 + GAMMA)
        nc.vector.tensor_scalar_mul(out=w_bm, in0=w_b, scalar1=-GAMMA)
    nc.vector.tensor_mul(
        kk2[0:NA, :], sk2[0:NA, 0:NB], sk2[0:NA, 0:NB]
    )
    nc.vector.tensor_mul(
        kk2[NA:P, :], sk2[NA:P, NB : 2 * NB], sk2[NA:P, NB : 2 * NB]
    )
    # diag_i expand of fp32r-rounded K for the C-phase AF*K matmul:
    # ra[i', i*64 + j] = K_r[i, j] if i == i' else 0, replicated in both
    # partition halves.
    nc.vector.tensor_copy(out=kk_r, in_=kk)
    nc.gpsimd.affine_select(
        out=ra[0:NA, :].rearrange("p (i j) -> p i j", i=NA),
        in_=kk_r[:, None, :].broadcast_to([NA, NA, NB]),
        compare_op=mybir.AluOpType.is_equal,
        fill=0.0,
        base=0,
        pattern=[[1, NA], [0, NB]],
        channel_multiplier=-1,
    )
    nc.sync.dma_start(out=ra[NA:P, :], in_=ra[0:NA, :])

    # ---- fixed-point iterations --------------------------------------
    # Step-interleaved emission: all chains' A-steps, then all B-steps.
    # Round 0 interleaves each chain's input transposes right before its
    # first A-step, so chain 0 starts iterating as soon as the first input
    # DMA half lands instead of after all 16 transposes.
    def chain_chunks(t):
        return [cc for cc in range(N_CHUNK) if chunk_map(cc)[1] == t]

    for s in range(n_rounds):
        last = s == n_rounds - 1
        for t in range(m_chains):
            if s == 0:
                for cc in chain_chunks(t):
                    tp_chunk(cc, "b")
                for cc in chain_chunks(t):
                    tp_chunk(cc, "a")
            ps1 = q_pool.tile([P, fd], F32, name=f"psA{s}_{t}", tag="q")
            if rich and last:
                # final A-step with the Richardson extrapolation folded in:
                # w_b @ ((1+g)*BF_k - g*BF_{k-1}) via two accumulating matmuls
                nc.tensor.matmul(ps1, w_bp, bf_c[t], start=True, stop=False)
                nc.tensor.matmul(ps1, w_bm, h1_c[t], start=False, stop=True)
            else:
                nc.tensor.matmul(ps1, w_b, bf_read(s, t), start=True, stop=True)
            r1 = r_pool.tile([P, fd], F32, tag="r")
            _act_recip(nc, r1, ps1, bias=1.0)
            nc.vector.tensor_mul(af_c[t], at_tc[t], r1)
            if last:
                # fp32r AF* for the C-phase expand
                nc.vector.tensor_copy(out=afr_c[t], in_=af_c[t])

        if last:
            break

        for t in range(m_chains):
            ps2 = q_pool.tile([P, fd], F32, name=f"psB{s}_{t}", tag="q")
            nc.tensor.matmul(ps2, w_a, af_c[t], start=True, stop=True)
            r2 = r_pool.tile([P, fd], F32, tag="r")
            _act_recip(nc, r2, ps2, bias=1.0)
            nc.vector.tensor_mul(bf_write(s, t), bt_tc[t], r2)

    # ---- C phase ------------------------------------------------------
    # Per chunk: batch-layout BF* (psb = AF*@K, recip, * BT), then the
    # quarter stream: E = AF*.K expand on PE, BF* broadcast multiply on the
    # engine given by qpat, fp16 DMA out.
    NQ = 4          # quarters per chunk
    QW = NA * NB // NQ                   # 1024 elements per quarter
    ni = QW // NB                        # i-values per quarter (16)
    for cc in range(N_CHUNK):
        g, t, col = chunk_map(cc)
        half = slice(g * NA, (g + 1) * NA)
        coff = slice(col, col + P)
        psb = aux_pool.tile([P, NB], F32, name=f"psb{cc}", tag="aux")
        nc.tensor.matmul(
            psb, af_c[t][half, coff], kk2[half, :], start=True, stop=True
        )
        rb = r_pool.tile([P, NB], F32, tag="r")
        _act_recip(nc, rb, psb, bias=1.0)
        # on GpSimd: all-SBUF op, keeps DVE free for the quarter multiplies
        nc.gpsimd.tensor_mul(bfs16_c[cc], bt_b[:, cc * NB : (cc + 1) * NB], rb)
        bfs_bc = bfs16_c[cc][:, None, :].broadcast_to([P, ni, NB])

        for q in range(NQ):
            kind = qpat[(cc * NQ + q) % len(qpat)]
            qp = q_pool.tile([P, QW], F32, tag="q")
            for h in range(2):
                nsl = slice(q * QW + h * 512, q * QW + (h + 1) * 512)
                nc.tensor.matmul(
                    qp[:, h * 512 : (h + 1) * 512],
                    afr_c[t][half, coff], ra[half, nsl],
                    start=True, stop=True,
                )
            cs = c_pool.tile([P, QW], F16, tag="c")
            if kind == "D":
                nc.vector.tensor_mul(
                    cs.rearrange("p (i j) -> p i j", i=ni),
                    qp.rearrange("p (i j) -> p i j", i=ni),
                    bfs_bc,
                )
            else:
                e16 = e_pool.tile([P, QW], F16, tag="e")
                _act_copy(nc, e16, qp)
                eng = nc.vector if kind == "A" else nc.gpsimd
                eng.tensor_mul(
                    cs.rearrange("p (i j) -> p i j", i=ni),
                    e16.rearrange("p (i j) -> p i j", i=ni),
                    bfs_bc,
                )
            nc.sync.dma_start(
                out=c_out[cc * P : (cc + 1) * P, q * QW : (q + 1) * QW], in_=cs
            )


def build_nc(n_solve=N_SOLVE, m_chains=M_CHAINS, t_repeat=1, timing_mode=False,
             rich=None, qpat=None):
    if rich is None:
        rich = RICH
    if qpat is None:
        qpat = QPAT
    nc = bacc.Bacc("TRN2", target_bir_lowering=False, debug=False, num_devices=N_CORES)
    at = nc.dram_tensor("at", (B_CORE, NA), F32, kind="ExternalInput").ap()
    bt = nc.dram_tensor("bt", (B_CORE, NB), F32, kind="ExternalInput").ap()
    sqk = nc.dram_tensor("sqk", (NA, NB), F32, kind="ExternalInput").ap()
    with tile.TileContext(nc) as tc:
        if timing_mode:
            tok = nc.dram_tensor("tok", (1, NA), F16, kind="ExternalOutput").ap()
            with ExitStack() as octx:
                dram = octx.enter_context(
                    tc.tile_pool(name="cdram", bufs=1, space="DRAM")
                )
                c = dram.tile([B_CORE, NA * NB], F16, tag="cscratch")
                for _ in range(t_repeat):
                    with ExitStack() as ctx:
                        _emit_core(ctx, tc, at, bt, sqk, c, n_solve, m_chains,
                                   rich, qpat)
                nc.sync.dma_start(out=tok, in_=c[0:1, 0:NA])
        else:
            c = nc.dram_tensor(
                "c", (B_CORE, NA * NB), F16, kind="ExternalOutput"
            ).ap()
            for _ in range(t_repeat):
                with ExitStack() as ctx:
                    _emit_core(ctx, tc, at, bt, sqk, c, n_solve, m_chains,
                               rich, qpat)
    nc.compile()
    return nc


_NC_CACHE = {}


def _get_nc(**kw):
    key = tuple(sorted(kw.items()))
    if key not in _NC_CACHE:
        _NC_CACHE[key] = build_nc(**kw)
    return _NC_CACHE[key]


def kernel(AT, BT, sqrt_K):
    AT = np.ascontiguousarray(AT, dtype=np.float32)
    BT = np.ascontiguousarray(BT, dtype=np.float32)
    sqrt_K = np.ascontiguousarray(sqrt_K, dtype=np.float32)
    nc = _get_nc(n_solve=N_SOLVE, m_chains=M_CHAINS)
    in_maps = [
        {
            "at": AT[c * B_CORE : (c + 1) * B_CORE],
            "bt": BT[c * B_CORE : (c + 1) * B_CORE],
            "sqk": sqrt_K,
        }
        for c in range(N_CORES)
    ]
    res = run_bass_kernel_spmd(nc, in_maps, core_ids=list(range(N_CORES)))
    return np.concatenate(
        [r["c"].astype(np.float32).reshape(B_CORE, NA, NB) for r in res.results],
        axis=0,
    )


# revision 49
# speedup vs baseline: 1.7374x; 1.0045x over previous
"""Trainium2 Bass kernel for nn_CompetitiveLayer_2 (competitive equilibrium layer).

Reference computation (per batch row b):
    K = sqrt_K ** 2                                  # (64, 64)
    repeat 30x:  AF = AT / (1 + BF @ K.T);  BF = BT / (1 + AF @ K)
    one more:    AF = AT / (1 + BF @ K.T);  BF = BT / (1 + AF @ K)
    C[b, i, j] = AF[b, i] * K[i, j] * BF[b, j]       # (B, 64, 64)

Sharding: pure data parallel over the batch dim, 1024 rows per core on 8 cores.

Per-core design:
  - State kept TRANSPOSED and 2-group packed: X_T[g*64 + j, col] = X[b, j]
    with b = (2*bl + g)*128 + p, col = bl*128 + p.  Both 64-row groups live in
    one 128-partition tile; the group-local matmul uses a block-diagonal
    [128, 128] stationary operand.
  - Each update is a serial chain (PE matmul -> ScalarE reciprocal LUT with
    bias=1 -> DVE multiply); the 512 batch columns split into M_CHAINS
    independent chains that pipeline across engines, step-interleaved.
    A dummy reciprocal at kernel start pulls the ~1.3us ACT table load
    under the input DMA.
  - Rounds: A_PRE plain rounds, then a scalar Richardson extrapolation
    BF* ~= BF_k + GAMMA*(BF_k - BF_{k-1}) (GAMMA = lam/(1-lam) for the
    fixed-point contraction lam~0.52; 2 DVE ops per chain vs 11+recip for
    full Aitken), then the final differentiable A-step.  The final BF* is
    produced per 128-row chunk directly in BATCH layout (psb = AF*@K via
    lhsT = transposed AF* chunk), so the transposed final B-step is
    dropped.  A_PRE=5 + Richardson ~ err 1.4e-3 vs the 2e-2 tolerance;
    the fp32r/fp16 C path adds ~1.5e-3 worst case.
  - C phase (per chunk, per 1024-element quarter): PE computes
    E[b, (i,j)] = AF*[b,i]*K[i,j] as a single fp32r matmul against the
    diag_i-expanded K (ra[i', i*64+j] = K[i,j] if i==i'), then the BF*
    broadcast multiply is spread across three engines to sit at the DMA
    write floor (~23us for 8 MB of fp16 C per core):
      direct quarters:  DVE  cs_fp16 = qp(PSUM f32) * bfs16-broadcast (1x)
      assist quarters:  ACT casts qp -> fp16 SBUF; DVE multiplies at 2x
      pool quarters:    ACT casts qp -> fp16 SBUF; GpSimd multiplies
    (broadcasting BF*[b,j] along i keeps the packed j dim innermost, which
    is what enables the 2x DVE mode).  C is written to DRAM as fp16; the
    host casts back to fp32 on gather.
"""

from contextlib import ExitStack

import numpy as np

import concourse.bass as bass
import concourse.tile as tile
from concourse import bacc, mybir
from concourse.bass_utils import run_bass_kernel_spmd
from concourse.masks import make_identity

F32 = mybir.dt.float32
F32R = mybir.dt.float32r
F16 = mybir.dt.float16
RECIP = mybir.ActivationFunctionType.Reciprocal


def _act_recip(nc, out, in_, bias=1.0):
    """out = 1 / (in_ + bias) on ScalarE.

    Emits InstActivation directly: nc.scalar.activation() refuses Reciprocal
    because of its LUT accuracy (~1.2e-5 rel, HW-measured), which is fine for
    this kernel's domain (inputs in [1, 22]) and tolerance.
    """
    eng = nc.scalar
    ins = [eng.lower_ap(in_)]
    for arg in (bias, 1.0, 0.0):  # bias, scale, alpha
        ins.append(mybir.ImmediateValue(dtype=mybir.dt.float32, value=float(arg)))
    return eng.add_instruction(
        mybir.InstActivation(
            name=nc.get_next_instruction_name(),
            func=RECIP,
            ins=ins,
            outs=[eng.lower_ap(out)],
        )
    )


def _act_copy(nc, out, in_):
    """out = in_ (dtype cast at write) on ScalarE via the Copy LUT."""
    eng = nc.scalar
    ins = [eng.lower_ap(in_)]
    for arg in (0.0, 1.0, 0.0):  # bias, scale, alpha
        ins.append(mybir.ImmediateValue(dtype=mybir.dt.float32, value=float(arg)))
    return eng.add_instruction(
        mybir.InstActivation(
            name=nc.get_next_instruction_name(),
            func=mybir.ActivationFunctionType.Copy,
            ins=ins,
            outs=[eng.lower_ap(out)],
        )
    )


P = 128          # SBUF partitions
NA = 64          # AF feature dim (i)
NB = 64          # BF feature dim (j)
B_TOTAL = 8192
N_CORES = 8
B_CORE = B_TOTAL // N_CORES          # 1024
N_CHUNK = B_CORE // P                # 8 output chunks of 128 rows
GROUPS = 2                           # partition-packing groups
COLS = B_CORE // GROUPS              # 512 batch columns per group
N_SOLVE = 10                         # plain solver iterations when RICH off
RICH = True                          # Richardson extrapolation after A_PRE rounds
A_PRE = 5                            # plain rounds before extrapolation
GAMMA = 1.10                         # Richardson coefficient lam/(1-lam)
M_CHAINS = 4                         # independent pipeline chains
FD = COLS // M_CHAINS                # free dim per chain (128)
# C-phase quarter engine assignment, cycled per (chunk, quarter):
# D = direct DVE (PSUM f32, 1x), A = ACT-cast + 2x DVE, G = ACT-cast + GpSimd
QPAT = ["D", "A", "G", "D"]


def _emit_core(ctx, tc, at, bt, sqk, c_out, n_solve, m_chains, rich,
               qpat=QPAT):
    """Emit the per-core kernel body into TileContext tc.

    at, bt: DRAM APs [1024, 64]; sqk: [64, 64]; c_out: [1024, 4096] fp16.
    """
    nc = tc.nc
    fd = COLS // m_chains
    if rich:
        n_pre = A_PRE
        n_rounds = n_pre + 1  # +1 = the final differentiable A-step
    else:
        n_pre = None
        n_rounds = n_solve + 1
    bpc = fd // P  # 128-col blocks per chain

    def chunk_map(cc):
        # chunk cc of 128 batch rows -> (group half, chain, col off)
        g, bl = cc % GROUPS, cc // GROUPS
        return g, bl // bpc, (bl % bpc) * P

    singles = ctx.enter_context(tc.tile_pool(name="singles", bufs=1))
    # PSUM budget is 8 banks.  One 3-buf pool of 2-bank tiles serves both the
    # iteration matmul outputs and the C-phase qp quarters: 3 bufs is enough
    # for the ACT-saturated iteration (buffer-reuse latency ~830ns < 3 recips
    # = 876ns) and puts the C-phase cadence (~(mm + cast + 2 sems)/3 = 620ns)
    # under the 728ns/quarter DMA floor.  A 2x1-bank aux pool holds the
    # setup transposes and the batch-B psb outputs.
    q_pool = ctx.enter_context(tc.tile_pool(name="qps", bufs=3, space="PSUM"))
    aux_pool = ctx.enter_context(tc.tile_pool(name="aux", bufs=2, space="PSUM"))
    r_pool = ctx.enter_context(tc.tile_pool(name="rp", bufs=8))
    e_pool = ctx.enter_context(tc.tile_pool(name="ep", bufs=6))
    c_pool = ctx.enter_context(tc.tile_pool(name="cp", bufs=10))

    # ---- static tiles -------------------------------------------------
    warm = singles.tile([1, 8], F32, tag="warm")
    # dummy reciprocal: forces the ACT Reciprocal table load at t=0 so the
    # ~1.3us LoadActFuncSet overlaps the input DMA instead of the first round
    nc.vector.memset(warm, 1.0)
    _act_recip(nc, warm, warm, bias=1.0)

    ident = singles.tile([P, P], F32, tag="ident")
    make_identity(nc, ident)

    at_b = singles.tile([P, COLS], F32, tag="at_b")   # batch layout: free=(chunk, i)
    bt_b = singles.tile([P, COLS], F32, tag="bt_b")
    at_tc = [
        singles.tile([P, fd], F32, name=f"at_t{t}", tag=f"at_t{t}")
        for t in range(m_chains)
    ]
    bt_tc = [
        singles.tile([P, fd], F32, name=f"bt_t{t}", tag=f"bt_t{t}")
        for t in range(m_chains)
    ]

    sk2 = singles.tile([P, 2 * NB], F32, tag="sk2")   # sqrt_K in both diag blocks
    kk = singles.tile([NA, NB], F32, tag="kk")        # K = sqrt_K^2   [i, j]
    w_a = singles.tile([P, P], F32, tag="w_a")        # blockdiag(K, K)
    w_b = singles.tile([P, P], F32, tag="w_b")        # blockdiag(K^T, K^T)
    # Richardson folded into the final A-step: w_b @ ((1+g)*BF_k - g*BF_{k-1})
    # as two PSUM-accumulating matmuls against pre-scaled stationaries.
    w_bp = singles.tile([P, P], F32, tag="w_bp")      # (1+GAMMA) * w_b
    w_bm = singles.tile([P, P], F32, tag="w_bm")      # -GAMMA * w_b
    kk2 = singles.tile([P, NB], F32, tag="kk2")       # K in both halves [i, j]
    kk_r = singles.tile([NA, NB], F32R, tag="kk_r")
    ra = singles.tile([P, NA * NB], F32R, tag="ra")   # diag_i-expanded K

    af_c = [singles.tile([P, fd], F32, name=f"af{t}", tag=f"af{t}") for t in range(m_chains)]
    bf_c = [singles.tile([P, fd], F32, name=f"bf{t}", tag=f"bf{t}") for t in range(m_chains)]
    afr_c = [
        singles.tile([P, fd], F32R, name=f"afr{t}", tag=f"afr{t}")
        for t in range(m_chains)
    ]
    bfs16_c = [
        singles.tile([P, NB], F16, name=f"bfs16_{cc}", tag=f"bfs16_{cc}")
        for cc in range(N_CHUNK)
    ]

    if rich:
        h1_c = [
            singles.tile([P, fd], F32, name=f"h1{t}", tag=f"h1{t}")
            for t in range(m_chains)
        ]
        hist = {n_pre - 2: h1_c}
    else:
        hist = {}

    def bf_read(s, t):
        # BF state entering round s's A-step for chain t
        if s == 0:
            return bt_tc[t]
        if (s - 1) in hist:
            return hist[s - 1][t]
        return bf_c[t]

    def bf_write(s, t):
        if s in hist:
            return hist[s][t]
        return bf_c[t]

    # ---- load inputs --------------------------------------------------
    # sqrt_K twice, once per diagonal block, so the block-diagonal weights
    # build with elementwise ops only (no serial SBUF->SBUF partition-shift
    # DMAs on the critical path to round 0).
    at3 = at.rearrange("(c p) i -> p c i", p=P)
    bt3 = bt.rearrange("(c p) i -> p c i", p=P)
    at_bv = at_b.rearrange("p (c i) -> p c i", i=NA)
    bt_bv = bt_b.rearrange("p (c i) -> p c i", i=NB)
    # sqrt_K first (tiny, gates the w_b build), then per-chain-pair input
    # slices with bt of chain t just ahead of at of chain t, so round 0's
    # chains come up at the steady recip cadence instead of waiting for
    # half-batch transfers.
    nc.sync.dma_start(out=sk2[0:NA, 0:NB], in_=sqk)
    nc.sync.dma_start(out=sk2[NA:P, NB : 2 * NB], in_=sqk)
    nbc = N_CHUNK // m_chains
    for t in range(m_chains):
        csl = slice(t * nbc, (t + 1) * nbc)
        nc.sync.dma_start(out=bt_bv[:, csl, :], in_=bt3[:, csl, :])
        nc.sync.dma_start(out=at_bv[:, csl, :], in_=at3[:, csl, :])

    # ---- chain 0's bt transposes + K build, critical-path ordered ------
    # PE FIFO: chain 0's bt transposes first (gated on the bt half-1 DMA),
    # then the w_b transposes (gated on kk <- sk2); everything else defers.
    def tp_chunk(cc, which):
        g, t, col = chunk_map(cc)
        tpi = aux_pool.tile([P, 2 * P], F32, name=f"tp{cc}{which}", tag="aux")
        if which == "b":
            tp2 = tpi[0:NB, 0:P]
            nc.tensor.transpose(tp2, bt_b[:, cc * NB : (cc + 1) * NB], ident)
            nc.vector.tensor_copy(
                out=bt_tc[t][g * NB : (g + 1) * NB, col : col + P], in_=tp2
            )
        else:
            tp1 = tpi[0:NA, P : 2 * P]
            nc.tensor.transpose(tp1, at_b[:, cc * NA : (cc + 1) * NA], ident)
            nc.vector.tensor_copy(
                out=at_tc[t][g * NA : (g + 1) * NA, col : col + P], in_=tp1
            )

    nc.vector.tensor_mul(kk, sk2[0:NA, 0:NB], sk2[0:NA, 0:NB])
    # K^T once on PE (transpose outputs must start at PSUM partition 0),
    # then copy into both diagonal blocks (DVE copies handle the partition
    # offset, same as the bt_tc/at_tc group copies)
    wps = aux_pool.tile([P, 2 * P], F32, tag="aux")
    nc.tensor.transpose(wps[0:NB, 0:NA], kk, ident[0:NA, 0:NA])
    nc.vector.memset(w_b, 0.0)
    nc.vector.tensor_copy(out=w_b[0:NB, 0:NA], in_=wps[0:NB, 0:NA])
    nc.vector.tensor_copy(out=w_b[NB:P, NA : 2 * NA], in_=wps[0:NB, 0:NA])

    # off the round-0 critical path: B-step / extrapolation / batch-B /
    # C-phase constants
    nc.vector.memset(w_a, 0.0)
    nc.vector.tensor_mul(
        w_a[0:NA, 0:NB], sk2[0:NA, 0:NB], sk2[0:NA, 0:NB]
    )
    nc.vector.tensor_mul(
        w_a[NA:P, NB : 2 * NB], sk2[NA:P, NB : 2 * NB], sk2[NA:P, NB : 2 * NB]
    )
    if rich:
        nc.vector.tensor_scalar_mul(out=w_bp, in0=w_b, scalar1=1.0 + GAMMA)
        nc.vector.tensor_scalar_mul(out=w_bm, in0=w_b, scalar1=-GAMMA)
    nc.vector.tensor_mul(
        kk2[0:NA, :], sk2[0:NA, 0:NB], sk2[0:NA, 0:NB]
    )
    nc.vector.tensor_mul(
        kk2[NA:P, :], sk2[NA:P, NB : 2 * NB], sk2[NA:P, NB : 2 * NB]
    )
    # diag_i expand of fp32r-rounded K for the C-phase AF*K matmul:
    # ra[i', i*64 + j] = K_r[i, j] if i == i' else 0, replicated in both
    # partition halves.
    nc.vector.tensor_copy(out=kk_r, in_=kk)
    nc.gpsimd.affine_select(
        out=ra[0:NA, :].rearrange("p (i j) -> p i j", i=NA),
        in_=kk_r[:, None, :].broadcast_to([NA, NA, NB]),
        compare_op=mybir.AluOpType.is_equal,
        fill=0.0,
        base=0,
        pattern=[[1, NA], [0, NB]],
        channel_multiplier=-1,
    )
    nc.sync.dma_start(out=ra[NA:P, :], in_=ra[0:NA, :])

    # ---- fixed-point iterations --------------------------------------
    # Step-interleaved emission: all chains' A-steps, then all B-steps.
    # Round 0 interleaves each chain's input transposes right before its
    # first A-step, so chain 0 starts iterating as soon as the first input
    # DMA half lands instead of after all 16 transposes.
    def chain_chunks(t):
        return [cc for cc in range(N_CHUNK) if chunk_map(cc)[1] == t]

    for s in range(n_rounds):
        last = s == n_rounds - 1
        for t in range(m_chains):
            if s == 0:
                for cc in chain_chunks(t):
                    tp_chunk(cc, "b")
                for cc in chain_chunks(t):
                    tp_chunk(cc, "a")
            ps1 = q_pool.tile([P, fd], F32, name=f"psA{s}_{t}", tag="q")
            if rich and last:
                # final A-step with the Richardson extrapolation folded in:
                # w_b @ ((1+g)*BF_k - g*BF_{k-1}) via two accumulating matmuls
                nc.tensor.matmul(ps1, w_bp, bf_c[t], start=True, stop=False)
                nc.tensor.matmul(ps1, w_bm, h1_c[t], start=False, stop=True)
            else:
                nc.tensor.matmul(ps1, w_b, bf_read(s, t), start=True, stop=True)
            r1 = r_pool.tile([P, fd], F32, tag="r")
            _act_recip(nc, r1, ps1, bias=1.0)
            nc.vector.tensor_mul(af_c[t], at_tc[t], r1)
            if last:
                # fp32r AF* for the C-phase expand
                nc.vector.tensor_copy(out=afr_c[t], in_=af_c[t])

        if last:
            break

        for t in range(m_chains):
            ps2 = q_pool.tile([P, fd], F32, name=f"psB{s}_{t}", tag="q")
            nc.tensor.matmul(ps2, w_a, af_c[t], start=True, stop=True)
            r2 = r_pool.tile([P, fd], F32, tag="r")
            _act_recip(nc, r2, ps2, bias=1.0)
            nc.vector.tensor_mul(bf_write(s, t), bt_tc[t], r2)

    # ---- C phase ------------------------------------------------------
    # Per chunk: batch-layout BF* (psb = AF*@K, recip, * BT), then the
    # quarter stream: E = AF*.K expand on PE, BF* broadcast multiply on the
    # engine given by qpat, fp16 DMA out.
    NQ = 4          # quarters per chunk
    QW = NA * NB // NQ                   # 1024 elements per quarter
    ni = QW // NB                        # i-values per quarter (16)
    for cc in range(N_CHUNK):
        g, t, col = chunk_map(cc)
        half = slice(g * NA, (g + 1) * NA)
        coff = slice(col, col + P)
        psb = aux_pool.tile([P, NB], F32, name=f"psb{cc}", tag="aux")
        nc.tensor.matmul(
            psb, af_c[t][half, coff], kk2[half, :], start=True, stop=True
        )
        rb = r_pool.tile([P, NB], F32, tag="r")
        _act_recip(nc, rb, psb, bias=1.0)
        # on GpSimd: all-SBUF op, keeps DVE free for the quarter multiplies
        nc.gpsimd.tensor_mul(bfs16_c[cc], bt_b[:, cc * NB : (cc + 1) * NB], rb)
        bfs_bc = bfs16_c[cc][:, None, :].broadcast_to([P, ni, NB])

        for q in range(NQ):
            kind = qpat[(cc * NQ + q) % len(qpat)]
            qp = q_pool.tile([P, QW], F32, tag="q")
            for h in range(2):
                nsl = slice(q * QW + h * 512, q * QW + (h + 1) * 512)
                nc.tensor.matmul(
                    qp[:, h * 512 : (h + 1) * 512],
                    afr_c[t][half, coff], ra[half, nsl],
                    start=True, stop=True,
                )
            cs = c_pool.tile([P, QW], F16, tag="c")
            if kind == "D":
                nc.vector.tensor_mul(
                    cs.rearrange("p (i j) -> p i j", i=ni),
                    qp.rearrange("p (i j) -> p i j", i=ni),
                    bfs_bc,
                )
            else:
                e16 = e_pool.tile([P, QW], F16, tag="e")
                _act_copy(nc, e16, qp)
                eng = nc.vector if kind == "A" else nc.gpsimd
                eng.tensor_mul(
                    cs.rearrange("p (i j) -> p i j", i=ni),
                    e16.rearrange("p (i j) -> p i j", i=ni),
                    bfs_bc,
                )
            nc.sync.dma_start(
                out=c_out[cc * P : (cc + 1) * P, q * QW : (q + 1) * QW], in_=cs
            )


def build_nc(n_solve=N_SOLVE, m_chains=M_CHAINS, t_repeat=1, timing_mode=False,
             rich=None, qpat=None):
    if rich is None:
        rich = RICH
    if qpat is None:
        qpat = QPAT
    nc = bacc.Bacc("TRN2", target_bir_lowering=False, debug=False, num_devices=N_CORES)
    at = nc.dram_tensor("at", (B_CORE, NA), F32, kind="ExternalInput").ap()
    bt = nc.dram_tensor("bt", (B_CORE, NB), F32, kind="ExternalInput").ap()
    sqk = nc.dram_tensor("sqk", (NA, NB), F32, kind="ExternalInput").ap()
    with tile.TileContext(nc) as tc:
        if timing_mode:
            tok = nc.dram_tensor("tok", (1, NA), F16, kind="ExternalOutput").ap()
            with ExitStack() as octx:
                dram = octx.enter_context(
                    tc.tile_pool(name="cdram", bufs=1, space="DRAM")
                )
                c = dram.tile([B_CORE, NA * NB], F16, tag="cscratch")
                for _ in range(t_repeat):
                    with ExitStack() as ctx:
                        _emit_core(ctx, tc, at, bt, sqk, c, n_solve, m_chains,
                                   rich, qpat)
                nc.sync.dma_start(out=tok, in_=c[0:1, 0:NA])
        else:
            c = nc.dram_tensor(
                "c", (B_CORE, NA * NB), F16, kind="ExternalOutput"
            ).ap()
            for _ in range(t_repeat):
                with ExitStack() as ctx:
                    _emit_core(ctx, tc, at, bt, sqk, c, n_solve, m_chains,
                               rich, qpat)
    nc.compile()
    return nc


_NC_CACHE = {}


def _get_nc(**kw):
    key = tuple(sorted(kw.items()))
    if key not in _NC_CACHE:
        _NC_CACHE[key] = build_nc(**kw)
    return _NC_CACHE[key]


def kernel(AT, BT, sqrt_K):
    AT = np.ascontiguousarray(AT, dtype=np.float32)
    BT = np.ascontiguousarray(BT, dtype=np.float32)
    sqrt_K = np.ascontiguousarray(sqrt_K, dtype=np.float32)
    nc = _get_nc(n_solve=N_SOLVE, m_chains=M_CHAINS)
    in_maps = [
        {
            "at": AT[c * B_CORE : (c + 1) * B_CORE],
            "bt": BT[c * B_CORE : (c + 1) * B_CORE],
            "sqk": sqrt_K,
        }
        for c in range(N_CORES)
    ]
    res = run_bass_kernel_spmd(nc, in_maps, core_ids=list(range(N_CORES)))
    return np.concatenate(
        [r["c"].astype(np.float32).reshape(B_CORE, NA, NB) for r in res.results],
        axis=0,
    )


# revision 50
# speedup vs baseline: 1.8152x; 1.0448x over previous
"""Trainium2 Bass kernel for nn_CompetitiveLayer_2 (competitive equilibrium layer).

Reference computation (per batch row b):
    K = sqrt_K ** 2                                  # (64, 64)
    repeat 30x:  AF = AT / (1 + BF @ K.T);  BF = BT / (1 + AF @ K)
    one more:    AF = AT / (1 + BF @ K.T);  BF = BT / (1 + AF @ K)
    C[b, i, j] = AF[b, i] * K[i, j] * BF[b, j]       # (B, 64, 64)

Sharding: pure data parallel over the batch dim, 1024 rows per core on 8 cores.

Per-core design:
  - State kept TRANSPOSED and 2-group packed: X_T[g*64 + j, col] = X[b, j]
    with b = (2*bl + g)*128 + p, col = bl*128 + p.  Both 64-row groups live in
    one 128-partition tile; the group-local matmul uses a block-diagonal
    [128, 128] stationary operand.
  - Each update is a serial chain (PE matmul -> ScalarE reciprocal LUT with
    bias=1 -> DVE multiply); the 512 batch columns split into M_CHAINS
    independent chains that pipeline across engines, step-interleaved.
    A dummy reciprocal at kernel start pulls the ~1.3us ACT table load
    under the input DMA.
  - Rounds: A_PRE plain rounds, then a scalar Richardson extrapolation
    BF* ~= BF_k + GAMMA*(BF_k - BF_{k-1}) (GAMMA = lam/(1-lam) for the
    fixed-point contraction lam~0.52; 2 DVE ops per chain vs 11+recip for
    full Aitken), then the final differentiable A-step.  The final BF* is
    produced per 128-row chunk directly in BATCH layout (psb = AF*@K via
    lhsT = transposed AF* chunk), so the transposed final B-step is
    dropped.  A_PRE=5 + Richardson ~ err 1.4e-3 vs the 2e-2 tolerance;
    the fp32r/fp16 C path adds ~1.5e-3 worst case.
  - C phase (per chunk, per 1024-element quarter): PE computes
    E[b, (i,j)] = AF*[b,i]*K[i,j] as a single fp32r matmul against the
    diag_i-expanded K (ra[i', i*64+j] = K[i,j] if i==i'), then the BF*
    broadcast multiply is spread across three engines to sit at the DMA
    write floor (~23us for 8 MB of fp16 C per core):
      direct quarters:  DVE  cs_fp16 = qp(PSUM f32) * bfs16-broadcast (1x)
      assist quarters:  ACT casts qp -> fp16 SBUF; DVE multiplies at 2x
      pool quarters:    ACT casts qp -> fp16 SBUF; GpSimd multiplies
    (broadcasting BF*[b,j] along i keeps the packed j dim innermost, which
    is what enables the 2x DVE mode).  C is written to DRAM as fp16; the
    host casts back to fp32 on gather.
"""

from contextlib import ExitStack

import numpy as np

import concourse.bass as bass
import concourse.tile as tile
from concourse import bacc, mybir
from concourse.bass_utils import run_bass_kernel_spmd
from concourse.masks import make_identity

F32 = mybir.dt.float32
F32R = mybir.dt.float32r
F16 = mybir.dt.float16
RECIP = mybir.ActivationFunctionType.Reciprocal


def _act_recip(nc, out, in_, bias=1.0):
    """out = 1 / (in_ + bias) on ScalarE.

    Emits InstActivation directly: nc.scalar.activation() refuses Reciprocal
    because of its LUT accuracy (~1.2e-5 rel, HW-measured), which is fine for
    this kernel's domain (inputs in [1, 22]) and tolerance.
    """
    eng = nc.scalar
    ins = [eng.lower_ap(in_)]
    for arg in (bias, 1.0, 0.0):  # bias, scale, alpha
        ins.append(mybir.ImmediateValue(dtype=mybir.dt.float32, value=float(arg)))
    return eng.add_instruction(
        mybir.InstActivation(
            name=nc.get_next_instruction_name(),
            func=RECIP,
            ins=ins,
            outs=[eng.lower_ap(out)],
        )
    )


def _act_copy(nc, out, in_):
    """out = in_ (dtype cast at write) on ScalarE via the Copy LUT."""
    eng = nc.scalar
    ins = [eng.lower_ap(in_)]
    for arg in (0.0, 1.0, 0.0):  # bias, scale, alpha
        ins.append(mybir.ImmediateValue(dtype=mybir.dt.float32, value=float(arg)))
    return eng.add_instruction(
        mybir.InstActivation(
            name=nc.get_next_instruction_name(),
            func=mybir.ActivationFunctionType.Copy,
            ins=ins,
            outs=[eng.lower_ap(out)],
        )
    )


P = 128          # SBUF partitions
NA = 64          # AF feature dim (i)
NB = 64          # BF feature dim (j)
B_TOTAL = 8192
N_CORES = 8
B_CORE = B_TOTAL // N_CORES          # 1024
N_CHUNK = B_CORE // P                # 8 output chunks of 128 rows
GROUPS = 2                           # partition-packing groups
COLS = B_CORE // GROUPS              # 512 batch columns per group
N_SOLVE = 10                         # plain solver iterations when RICH off
RICH = True                          # Richardson extrapolation after A_PRE rounds
A_PRE = 4                            # plain rounds before extrapolation
GAMMA = 1.05                         # Richardson coefficient lam/(1-lam)
M_CHAINS = 4                         # independent pipeline chains
FD = COLS // M_CHAINS                # free dim per chain (128)
# C-phase quarter engine assignment, cycled per (chunk, quarter):
# D = direct DVE (PSUM f32, 1x), A = ACT-cast + 2x DVE, G = ACT-cast + GpSimd
QPAT = ["D", "A", "G", "D"]


def _emit_core(ctx, tc, at, bt, sqk, c_out, n_solve, m_chains, rich,
               qpat=QPAT):
    """Emit the per-core kernel body into TileContext tc.

    at, bt: DRAM APs [1024, 64]; sqk: [64, 64]; c_out: [1024, 4096] fp16.
    """
    nc = tc.nc
    fd = COLS // m_chains
    if rich:
        n_pre = A_PRE
        n_rounds = n_pre + 1  # +1 = the final differentiable A-step
    else:
        n_pre = None
        n_rounds = n_solve + 1
    bpc = fd // P  # 128-col blocks per chain

    def chunk_map(cc):
        # chunk cc of 128 batch rows -> (group half, chain, col off)
        g, bl = cc % GROUPS, cc // GROUPS
        return g, bl // bpc, (bl % bpc) * P

    singles = ctx.enter_context(tc.tile_pool(name="singles", bufs=1))
    # PSUM budget is 8 banks.  One 3-buf pool of 2-bank tiles serves both the
    # iteration matmul outputs and the C-phase qp quarters: 3 bufs is enough
    # for the ACT-saturated iteration (buffer-reuse latency ~830ns < 3 recips
    # = 876ns) and puts the C-phase cadence (~(mm + cast + 2 sems)/3 = 620ns)
    # under the 728ns/quarter DMA floor.  A 2x1-bank aux pool holds the
    # setup transposes and the batch-B psb outputs.
    q_pool = ctx.enter_context(tc.tile_pool(name="qps", bufs=3, space="PSUM"))
    aux_pool = ctx.enter_context(tc.tile_pool(name="aux", bufs=2, space="PSUM"))
    r_pool = ctx.enter_context(tc.tile_pool(name="rp", bufs=8))
    e_pool = ctx.enter_context(tc.tile_pool(name="ep", bufs=6))
    c_pool = ctx.enter_context(tc.tile_pool(name="cp", bufs=10))

    # ---- static tiles -------------------------------------------------
    warm = singles.tile([1, 8], F32, tag="warm")
    # dummy reciprocal: forces the ACT Reciprocal table load at t=0 so the
    # ~1.3us LoadActFuncSet overlaps the input DMA instead of the first round
    nc.vector.memset(warm, 1.0)
    _act_recip(nc, warm, warm, bias=1.0)

    ident = singles.tile([P, P], F32, tag="ident")
    make_identity(nc, ident)

    at_b = singles.tile([P, COLS], F32, tag="at_b")   # batch layout: free=(chunk, i)
    bt_b = singles.tile([P, COLS], F32, tag="bt_b")
    at_tc = [
        singles.tile([P, fd], F32, name=f"at_t{t}", tag=f"at_t{t}")
        for t in range(m_chains)
    ]
    bt_tc = [
        singles.tile([P, fd], F32, name=f"bt_t{t}", tag=f"bt_t{t}")
        for t in range(m_chains)
    ]

    sk2 = singles.tile([P, 2 * NB], F32, tag="sk2")   # sqrt_K in both diag blocks
    kk = singles.tile([NA, NB], F32, tag="kk")        # K = sqrt_K^2   [i, j]
    w_a = singles.tile([P, P], F32, tag="w_a")        # blockdiag(K, K)
    w_b = singles.tile([P, P], F32, tag="w_b")        # blockdiag(K^T, K^T)
    # Richardson folded into the final A-step: w_b @ ((1+g)*BF_k - g*BF_{k-1})
    # as two PSUM-accumulating matmuls against pre-scaled stationaries.
    w_bp = singles.tile([P, P], F32, tag="w_bp")      # (1+GAMMA) * w_b
    w_bm = singles.tile([P, P], F32, tag="w_bm")      # -GAMMA * w_b
    kk2 = singles.tile([P, NB], F32, tag="kk2")       # K in both halves [i, j]
    kk_r = singles.tile([NA, NB], F32R, tag="kk_r")
    ra = singles.tile([P, NA * NB], F32R, tag="ra")   # diag_i-expanded K

    af_c = [singles.tile([P, fd], F32, name=f"af{t}", tag=f"af{t}") for t in range(m_chains)]
    bf_c = [singles.tile([P, fd], F32, name=f"bf{t}", tag=f"bf{t}") for t in range(m_chains)]
    afr_c = [
        singles.tile([P, fd], F32R, name=f"afr{t}", tag=f"afr{t}")
        for t in range(m_chains)
    ]
    bfs16_c = [
        singles.tile([P, NB], F16, name=f"bfs16_{cc}", tag=f"bfs16_{cc}")
        for cc in range(N_CHUNK)
    ]

    if rich:
        h1_c = [
            singles.tile([P, fd], F32, name=f"h1{t}", tag=f"h1{t}")
            for t in range(m_chains)
        ]
        hist = {n_pre - 2: h1_c}
    else:
        hist = {}

    def bf_read(s, t):
        # BF state entering round s's A-step for chain t
        if s == 0:
            return bt_tc[t]
        if (s - 1) in hist:
            return hist[s - 1][t]
        return bf_c[t]

    def bf_write(s, t):
        if s in hist:
            return hist[s][t]
        return bf_c[t]

    # ---- load inputs --------------------------------------------------
    # sqrt_K twice, once per diagonal block, so the block-diagonal weights
    # build with elementwise ops only (no serial SBUF->SBUF partition-shift
    # DMAs on the critical path to round 0).
    at3 = at.rearrange("(c p) i -> p c i", p=P)
    bt3 = bt.rearrange("(c p) i -> p c i", p=P)
    at_bv = at_b.rearrange("p (c i) -> p c i", i=NA)
    bt_bv = bt_b.rearrange("p (c i) -> p c i", i=NB)
    # sqrt_K first (tiny, gates the w_b build), then per-chain-pair input
    # slices with bt of chain t just ahead of at of chain t, so round 0's
    # chains come up at the steady recip cadence instead of waiting for
    # half-batch transfers.
    nc.sync.dma_start(out=sk2[0:NA, 0:NB], in_=sqk)
    nc.sync.dma_start(out=sk2[NA:P, NB : 2 * NB], in_=sqk)
    nbc = N_CHUNK // m_chains
    for t in range(m_chains):
        csl = slice(t * nbc, (t + 1) * nbc)
        nc.sync.dma_start(out=bt_bv[:, csl, :], in_=bt3[:, csl, :])
        nc.sync.dma_start(out=at_bv[:, csl, :], in_=at3[:, csl, :])

    # ---- chain 0's bt transposes + K build, critical-path ordered ------
    # PE FIFO: chain 0's bt transposes first (gated on the bt half-1 DMA),
    # then the w_b transposes (gated on kk <- sk2); everything else defers.
    def tp_chunk(cc, which):
        g, t, col = chunk_map(cc)
        tpi = aux_pool.tile([P, 2 * P], F32, name=f"tp{cc}{which}", tag="aux")
        if which == "b":
            tp2 = tpi[0:NB, 0:P]
            nc.tensor.transpose(tp2, bt_b[:, cc * NB : (cc + 1) * NB], ident)
            nc.vector.tensor_copy(
                out=bt_tc[t][g * NB : (g + 1) * NB, col : col + P], in_=tp2
            )
        else:
            tp1 = tpi[0:NA, P : 2 * P]
            nc.tensor.transpose(tp1, at_b[:, cc * NA : (cc + 1) * NA], ident)
            nc.vector.tensor_copy(
                out=at_tc[t][g * NA : (g + 1) * NA, col : col + P], in_=tp1
            )

    nc.vector.tensor_mul(kk, sk2[0:NA, 0:NB], sk2[0:NA, 0:NB])
    # K^T once on PE (transpose outputs must start at PSUM partition 0),
    # then copy into both diagonal blocks (DVE copies handle the partition
    # offset, same as the bt_tc/at_tc group copies)
    wps = aux_pool.tile([P, 2 * P], F32, tag="aux")
    nc.tensor.transpose(wps[0:NB, 0:NA], kk, ident[0:NA, 0:NA])
    nc.vector.memset(w_b, 0.0)
    nc.vector.tensor_copy(out=w_b[0:NB, 0:NA], in_=wps[0:NB, 0:NA])
    nc.vector.tensor_copy(out=w_b[NB:P, NA : 2 * NA], in_=wps[0:NB, 0:NA])

    # off the round-0 critical path: B-step / extrapolation / batch-B /
    # C-phase constants
    nc.vector.memset(w_a, 0.0)
    nc.vector.tensor_mul(
        w_a[0:NA, 0:NB], sk2[0:NA, 0:NB], sk2[0:NA, 0:NB]
    )
    nc.vector.tensor_mul(
        w_a[NA:P, NB : 2 * NB], sk2[NA:P, NB : 2 * NB], sk2[NA:P, NB : 2 * NB]
    )
    if rich:
        nc.vector.tensor_scalar_mul(out=w_bp, in0=w_b, scalar1=1.0 + GAMMA)
        nc.vector.tensor_scalar_mul(out=w_bm, in0=w_b, scalar1=-GAMMA)
    nc.vector.tensor_mul(
        kk2[0:NA, :], sk2[0:NA, 0:NB], sk2[0:NA, 0:NB]
    )
    nc.vector.tensor_mul(
        kk2[NA:P, :], sk2[NA:P, NB : 2 * NB], sk2[NA:P, NB : 2 * NB]
    )
    # diag_i expand of fp32r-rounded K for the C-phase AF*K matmul:
    # ra[i', i*64 + j] = K_r[i, j] if i == i' else 0, replicated in both
    # partition halves.
    nc.vector.tensor_copy(out=kk_r, in_=kk)
    nc.gpsimd.affine_select(
        out=ra[0:NA, :].rearrange("p (i j) -> p i j", i=NA),
        in_=kk_r[:, None, :].broadcast_to([NA, NA, NB]),
        compare_op=mybir.AluOpType.is_equal,
        fill=0.0,
        base=0,
        pattern=[[1, NA], [0, NB]],
        channel_multiplier=-1,
    )
    nc.sync.dma_start(out=ra[NA:P, :], in_=ra[0:NA, :])

    # ---- fixed-point iterations --------------------------------------
    # Step-interleaved emission: all chains' A-steps, then all B-steps.
    # Round 0 interleaves each chain's input transposes right before its
    # first A-step, so chain 0 starts iterating as soon as the first input
    # DMA half lands instead of after all 16 transposes.
    def chain_chunks(t):
        return [cc for cc in range(N_CHUNK) if chunk_map(cc)[1] == t]

    for s in range(n_rounds):
        last = s == n_rounds - 1
        for t in range(m_chains):
            if s == 0:
                for cc in chain_chunks(t):
                    tp_chunk(cc, "b")
                for cc in chain_chunks(t):
                    tp_chunk(cc, "a")
            ps1 = q_pool.tile([P, fd], F32, name=f"psA{s}_{t}", tag="q")
            if rich and last:
                # final A-step with the Richardson extrapolation folded in:
                # w_b @ ((1+g)*BF_k - g*BF_{k-1}) via two accumulating matmuls
                nc.tensor.matmul(ps1, w_bp, bf_c[t], start=True, stop=False)
                nc.tensor.matmul(ps1, w_bm, h1_c[t], start=False, stop=True)
            else:
                nc.tensor.matmul(ps1, w_b, bf_read(s, t), start=True, stop=True)
            r1 = r_pool.tile([P, fd], F32, tag="r")
            _act_recip(nc, r1, ps1, bias=1.0)
            nc.vector.tensor_mul(af_c[t], at_tc[t], r1)
            if last:
                # fp32r AF* for the C-phase expand
                nc.vector.tensor_copy(out=afr_c[t], in_=af_c[t])

        if last:
            break

        for t in range(m_chains):
            ps2 = q_pool.tile([P, fd], F32, name=f"psB{s}_{t}", tag="q")
            nc.tensor.matmul(ps2, w_a, af_c[t], start=True, stop=True)
            r2 = r_pool.tile([P, fd], F32, tag="r")
            _act_recip(nc, r2, ps2, bias=1.0)
            nc.vector.tensor_mul(bf_write(s, t), bt_tc[t], r2)

    # ---- C phase ------------------------------------------------------
    # Per chunk: batch-layout BF* (psb = AF*@K, recip, * BT), then the
    # quarter stream: E = AF*.K expand on PE, BF* broadcast multiply on the
    # engine given by qpat, fp16 DMA out.
    NQ = 4          # quarters per chunk
    QW = NA * NB // NQ                   # 1024 elements per quarter
    ni = QW // NB                        # i-values per quarter (16)
    for cc in range(N_CHUNK):
        g, t, col = chunk_map(cc)
        half = slice(g * NA, (g + 1) * NA)
        coff = slice(col, col + P)
        psb = aux_pool.tile([P, NB], F32, name=f"psb{cc}", tag="aux")
        nc.tensor.matmul(
            psb, af_c[t][half, coff], kk2[half, :], start=True, stop=True
        )
        rb = r_pool.tile([P, NB], F32, tag="r")
        _act_recip(nc, rb, psb, bias=1.0)
        # on GpSimd: all-SBUF op, keeps DVE free for the quarter multiplies
        nc.gpsimd.tensor_mul(bfs16_c[cc], bt_b[:, cc * NB : (cc + 1) * NB], rb)
        bfs_bc = bfs16_c[cc][:, None, :].broadcast_to([P, ni, NB])

        for q in range(NQ):
            kind = qpat[(cc * NQ + q) % len(qpat)]
            qp = q_pool.tile([P, QW], F32, tag="q")
            for h in range(2):
                nsl = slice(q * QW + h * 512, q * QW + (h + 1) * 512)
                nc.tensor.matmul(
                    qp[:, h * 512 : (h + 1) * 512],
                    afr_c[t][half, coff], ra[half, nsl],
                    start=True, stop=True,
                )
            cs = c_pool.tile([P, QW], F16, tag="c")
            if kind == "D":
                nc.vector.tensor_mul(
                    cs.rearrange("p (i j) -> p i j", i=ni),
                    qp.rearrange("p (i j) -> p i j", i=ni),
                    bfs_bc,
                )
            else:
                e16 = e_pool.tile([P, QW], F16, tag="e")
                _act_copy(nc, e16, qp)
                eng = nc.vector if kind == "A" else nc.gpsimd
                eng.tensor_mul(
                    cs.rearrange("p (i j) -> p i j", i=ni),
                    e16.rearrange("p (i j) -> p i j", i=ni),
                    bfs_bc,
                )
            nc.sync.dma_start(
                out=c_out[cc * P : (cc + 1) * P, q * QW : (q + 1) * QW], in_=cs
            )


def build_nc(n_solve=N_SOLVE, m_chains=M_CHAINS, t_repeat=1, timing_mode=False,
             rich=None, qpat=None):
    if rich is None:
        rich = RICH
    if qpat is None:
        qpat = QPAT
    nc = bacc.Bacc("TRN2", target_bir_lowering=False, debug=False, num_devices=N_CORES)
    at = nc.dram_tensor("at", (B_CORE, NA), F32, kind="ExternalInput").ap()
    bt = nc.dram_tensor("bt", (B_CORE, NB), F32, kind="ExternalInput").ap()
    sqk = nc.dram_tensor("sqk", (NA, NB), F32, kind="ExternalInput").ap()
    with tile.TileContext(nc) as tc:
        if timing_mode:
            tok = nc.dram_tensor("tok", (1, NA), F16, kind="ExternalOutput").ap()
            with ExitStack() as octx:
                dram = octx.enter_context(
                    tc.tile_pool(name="cdram", bufs=1, space="DRAM")
                )
                c = dram.tile([B_CORE, NA * NB], F16, tag="cscratch")
                for _ in range(t_repeat):
                    with ExitStack() as ctx:
                        _emit_core(ctx, tc, at, bt, sqk, c, n_solve, m_chains,
                                   rich, qpat)
                nc.sync.dma_start(out=tok, in_=c[0:1, 0:NA])
        else:
            c = nc.dram_tensor(
                "c", (B_CORE, NA * NB), F16, kind="ExternalOutput"
            ).ap()
            for _ in range(t_repeat):
                with ExitStack() as ctx:
                    _emit_core(ctx, tc, at, bt, sqk, c, n_solve, m_chains,
                               rich, qpat)
    nc.compile()
    return nc


_NC_CACHE = {}


def _get_nc(**kw):
    key = tuple(sorted(kw.items()))
    if key not in _NC_CACHE:
        _NC_CACHE[key] = build_nc(**kw)
    return _NC_CACHE[key]


def kernel(AT, BT, sqrt_K):
    AT = np.ascontiguousarray(AT, dtype=np.float32)
    BT = np.ascontiguousarray(BT, dtype=np.float32)
    sqrt_K = np.ascontiguousarray(sqrt_K, dtype=np.float32)
    nc = _get_nc(n_solve=N_SOLVE, m_chains=M_CHAINS)
    in_maps = [
        {
            "at": AT[c * B_CORE : (c + 1) * B_CORE],
            "bt": BT[c * B_CORE : (c + 1) * B_CORE],
            "sqk": sqrt_K,
        }
        for c in range(N_CORES)
    ]
    res = run_bass_kernel_spmd(nc, in_maps, core_ids=list(range(N_CORES)))
    return np.concatenate(
        [r["c"].astype(np.float32).reshape(B_CORE, NA, NB) for r in res.results],
        axis=0,
    )
